# revision 6
# baseline (speedup 1.0000x reference)
"""GCN (2-layer, PyG GCNConv-style) on 8 Trainium2 NeuronCores.

Sharding: nodes are assigned to 8*98 destination tiles of 128 rows with a
degree-balanced bin packing (so every tile carries ~the same edge count),
8*12544 padded rows total; the host undoes the permutation when reassembling
the output. Edges (incl. self-loops) carry the symmetric D^-1/2 A D^-1/2
norm. The host lays the scaled edge messages out contiguously per (core, dst
tile, 128-edge chunk), so every device-side access is a sequential stream
(the gather pattern is static data, so it is folded into the input
sharding). On device, each 128-edge chunk is segment-summed with one PE
matmul against a 0/1 one-hot built by a single DVE tensor_scalar (is_equal
against an iota row). By linearity segsum(h[src]*norm) = segsum(x[src]*norm)
@ W1, so the dense transforms run once per dst tile after aggregation.
Phase 1 computes h2 = relu(agg1@W1 + b1)@W2 per node shard; the host then
routes the h2 edge messages (the halo exchange) and phase 2 segment-sums
them (b2 is folded into one message slot per node).
"""

import heapq
import math
import os
import sys
import types

sys.path.insert(0, "/opt/trn_rl_repo")

import numpy as np

N_NODES = 100000
IN_DIM = 128
HID_DIM = 128
OUT_DIM = 64
N_CORES = 8
NPC = N_NODES // N_CORES          # nodes per core
TILES = math.ceil(NPC / 128)      # dst tiles per core
NPC_PAD = TILES * 128             # padded rows per core

LAST_RESULTS = []  # BassKernelResults of the most recent kernel() call


def _install_axon_ntff_hook():
    """Register the NTFF profiling hook that the stub antenv package lacks."""
    try:
        import antenv
        if getattr(antenv, "axon_hooks", None) is not None:
            return
        hooks_mod = types.ModuleType("antenv.axon_hooks")
        _hook = [None]
        hooks_mod.set_axon_ntff_profile_hook = lambda h: _hook.__setitem__(0, h)
        hooks_mod.get_axon_ntff_profile_hook = lambda: _hook[0]
        sys.modules["antenv.axon_hooks"] = hooks_mod
        antenv.axon_hooks = hooks_mod
        from trn_agent_boot.trn_boot import _ntff_profile_via_ctypes
        hooks_mod.set_axon_ntff_profile_hook(
            _ntff_profile_via_ctypes("/opt/axon/libaxon_pjrt.so")
        )
    except Exception:
        pass


def _edge_dt():
    import concourse.mybir as mybir
    return (mybir.dt.bfloat16
            if os.environ.get("BASSGCN_EDGE_DT", "bf16") == "bf16"
            else mybir.dt.float32)


def _edge_np():
    import ml_dtypes
    return (ml_dtypes.bfloat16
            if os.environ.get("BASSGCN_EDGE_DT", "bf16") == "bf16"
            else np.float32)


def build_phase1(K, *, in_dim=IN_DIM, hid_dim=HID_DIM, out_dim=OUT_DIM,
                 n_cores=N_CORES, tiles=TILES):
    """x-messages (pre-gathered, pre-scaled) -> h2 = relu(agg@W1+b1)@W2."""
    import concourse.bacc as bacc
    import concourse.mybir as mybir
    import concourse.tile as tile

    f32 = mybir.dt.float32
    edt = _edge_dt()
    TK = tiles * K

    nc = bacc.Bacc("TRN2", target_bir_lowering=False, debug=False,
                   num_devices=n_cores)
    xe = nc.dram_tensor("xe", [128, TK * in_dim], edt, kind="ExternalInput")
    dli = nc.dram_tensor("dstloc", [128, TK], f32, kind="ExternalInput")
    w1i = nc.dram_tensor("W1", [in_dim, hid_dim], f32, kind="ExternalInput")
    w2i = nc.dram_tensor("W2", [hid_dim, out_dim], f32, kind="ExternalInput")
    b1i = nc.dram_tensor("b1", [hid_dim, 1], f32, kind="ExternalInput")
    ioi = nc.dram_tensor("iota", [128, 128], edt, kind="ExternalInput")
    idi = nc.dram_tensor("ident", [out_dim, out_dim], f32, kind="ExternalInput")
    h2o = nc.dram_tensor("h2", [tiles * 128, out_dim], f32,
                         kind="ExternalOutput")

    is_eq = mybir.AluOpType.is_equal
    Relu = mybir.ActivationFunctionType.Relu

    with tile.TileContext(nc) as tc:
        with (
            tc.tile_pool(name="const", bufs=1) as cp,
            tc.tile_pool(name="gath", bufs=3) as gp,
            tc.tile_pool(name="oh", bufs=6) as ohp,
            tc.tile_pool(name="epi", bufs=3) as ep,
            tc.tile_pool(name="psum", bufs=2, space="PSUM") as pp,
        ):
            w1s = cp.tile([in_dim, hid_dim], f32, name="w1s")
            nc.sync.dma_start(out=w1s[:], in_=w1i[:, :])
            w2s = cp.tile([hid_dim, out_dim], f32, name="w2s")
            nc.sync.dma_start(out=w2s[:], in_=w2i[:, :])
            b1s = cp.tile([hid_dim, 1], f32, name="b1s")
            nc.sync.dma_start(out=b1s[:], in_=b1i[:, :])
            iotas = cp.tile([128, 128], edt, name="iotas")
            nc.sync.dma_start(out=iotas[:], in_=ioi[:, :])
            idents = cp.tile([out_dim, out_dim], f32, name="idents")
            nc.sync.dma_start(out=idents[:], in_=idi[:, :])
            dls = cp.tile([128, TK], f32, name="dls")
            nc.sync.dma_start(out=dls[:], in_=dli[:, :])

            for t in range(tiles):
                xg = gp.tile([128, K * in_dim], edt, name="xg", tag="xg")
                nc.sync.dma_start(
                    out=xg[:], in_=xe[:, t * K * in_dim:(t + 1) * K * in_dim])
                pA = pp.tile([128, 128], f32, name="pA", tag="acc")
                for g in range(K):
                    c = t * K + g
                    oh = ohp.tile([128, 128], edt, name="oh", tag="oh")
                    nc.vector.tensor_scalar(
                        out=oh[:], in0=iotas[:], scalar1=dls[:, c:c + 1],
                        scalar2=None, op0=is_eq)
                    nc.tensor.matmul(
                        out=pA[:], lhsT=xg[:, g * in_dim:(g + 1) * in_dim],
                        rhs=oh[:], start=(g == 0), stop=(g == K - 1))
                # aggxT[k, d] in pA; h1T = relu(W1^T @ aggxT + b1)
                aggxT = ep.tile([128, 128], f32, name="aggxT", tag="aggxT")
                nc.scalar.copy(out=aggxT[:], in_=pA[:])
                pB = pp.tile([128, 128], f32, name="pB", tag="fin")
                nc.tensor.matmul(out=pB[:], lhsT=w1s[:], rhs=aggxT[:],
                                 start=True, stop=True)
                h1T = ep.tile([128, 128], f32, name="h1T", tag="h1T")
                nc.scalar.activation(out=h1T[:], in_=pB[:], func=Relu,
                                     bias=b1s[:, 0:1], scale=1.0)
                # h2T = W2^T @ h1T (b2 is folded into the phase-2 messages)
                pC = pp.tile([out_dim, 128], f32, name="pC", tag="fin")
                nc.tensor.matmul(out=pC[:], lhsT=w2s[:], rhs=h1T[:],
                                 start=True, stop=True)
                h2T = ep.tile([out_dim, 128], f32, name="h2T", tag="h2T")
                nc.scalar.copy(out=h2T[:], in_=pC[:])
                pD = pp.tile([128, out_dim], f32, name="pD", tag="fin")
                nc.tensor.transpose(out=pD[:], in_=h2T[:], identity=idents[:])
                h2t = ep.tile([128, out_dim], f32, name="h2t", tag="h2t")
                nc.scalar.copy(out=h2t[:], in_=pD[:])
                nc.sync.dma_start(out=h2o[t * 128:(t + 1) * 128, :],
                                  in_=h2t[:])
    nc.compile()
    return nc


def build_phase2(K, *, out_dim=OUT_DIM, n_cores=N_CORES, tiles=TILES):
    """h2-messages (pre-gathered, pre-scaled, +b2 folded) -> out = segsum."""
    import concourse.bacc as bacc
    import concourse.mybir as mybir
    import concourse.tile as tile

    f32 = mybir.dt.float32
    edt = _edge_dt()
    TK = tiles * K

    nc = bacc.Bacc("TRN2", target_bir_lowering=False, debug=False,
                   num_devices=n_cores)
    he = nc.dram_tensor("he", [128, TK * out_dim], edt, kind="ExternalInput")
    dli = nc.dram_tensor("dstloc", [128, TK], f32, kind="ExternalInput")
    ioi = nc.dram_tensor("iota", [128, 128], edt, kind="ExternalInput")
    out_t = nc.dram_tensor("out", [tiles * 128, out_dim], f32,
                           kind="ExternalOutput")

    is_eq = mybir.AluOpType.is_equal

    with tile.TileContext(nc) as tc:
        with (
            tc.tile_pool(name="const", bufs=1) as cp,
            tc.tile_pool(name="gath", bufs=3) as gp,
            tc.tile_pool(name="oh", bufs=6) as ohp,
            tc.tile_pool(name="epi", bufs=3) as ep,
            tc.tile_pool(name="psum", bufs=2, space="PSUM") as pp,
        ):
            iotas = cp.tile([128, 128], edt, name="iotas")
            nc.sync.dma_start(out=iotas[:], in_=ioi[:, :])
            dls = cp.tile([128, TK], f32, name="dls")
            nc.sync.dma_start(out=dls[:], in_=dli[:, :])

            for t in range(tiles):
                hg = gp.tile([128, K * out_dim], edt, name="hg", tag="hg")
                nc.sync.dma_start(
                    out=hg[:], in_=he[:, t * K * out_dim:(t + 1) * K * out_dim])
                pE = pp.tile([128, out_dim], f32, name="pE", tag="acc")
                for g in range(K):
                    c = t * K + g
                    oh = ohp.tile([128, 128], edt, name="oh", tag="oh")
                    nc.vector.tensor_scalar(
                        out=oh[:], in0=iotas[:], scalar1=dls[:, c:c + 1],
                        scalar2=None, op0=is_eq)
                    # agg[d, o] += onehot[e, d]^T @ hg[e, o]
                    nc.tensor.matmul(
                        out=pE[:], lhsT=oh[:],
                        rhs=hg[:, g * out_dim:(g + 1) * out_dim],
                        start=(g == 0), stop=(g == K - 1))
                ot = ep.tile([128, out_dim], f32, name="ot", tag="ot")
                nc.scalar.copy(out=ot[:], in_=pE[:])
                nc.sync.dma_start(out=out_t[t * 128:(t + 1) * 128, :],
                                  in_=ot[:])
    nc.compile()
    return nc


def _balance_bins(deg, n_bins, cap=128):
    """Degree-balanced bin packing: each bin gets <=cap nodes, edge sums even.

    Returns (assign[node] -> bin, slot[node] -> row within bin)."""
    n = len(deg)
    order = np.argsort(-deg, kind="stable")
    heap = [(0, b) for b in range(n_bins)]
    heapq.heapify(heap)
    counts = np.zeros(n_bins, np.int32)
    assign = np.empty(n, np.int32)
    slot = np.empty(n, np.int32)
    for i in order:
        s, b = heapq.heappop(heap)
        assign[i] = b
        slot[i] = counts[b]
        counts[b] += 1
        if counts[b] < cap:
            heapq.heappush(heap, (s + int(deg[i]), b))
    return assign, slot


def shard_edges(edge_index, *, n_nodes=N_NODES, n_cores=N_CORES, tiles=TILES):
    """Balanced bucketing of dst nodes into (core, tile, row); edge slotting."""
    ei = np.asarray(edge_index)
    src = ei[0].astype(np.int64)
    dst = ei[1].astype(np.int64)

    loops = np.arange(n_nodes, dtype=np.int64)
    src_all = np.concatenate([src, loops])
    dst_all = np.concatenate([dst, loops])
    e_tot = src_all.shape[0]

    deg = np.bincount(dst_all, minlength=n_nodes).astype(np.int64)
    dis = 1.0 / np.sqrt(deg.astype(np.float64))  # self-loops => deg >= 1
    norm = (dis[src_all] * dis[dst_all]).astype(np.float32)

    n_bins = n_cores * tiles
    assign, slot = _balance_bins(deg, n_bins)

    # Balance bins across cores: snake-deal bins sorted by edge sum.
    bin_sums = np.bincount(assign, weights=deg, minlength=n_bins).astype(
        np.int64)
    border = np.argsort(-bin_sums, kind="stable")
    bin_core = np.empty(n_bins, np.int32)
    bin_tile = np.empty(n_bins, np.int32)
    tile_ctr = np.zeros(n_cores, np.int32)
    for r, b in enumerate(border):
        rr = r // n_cores
        c = (r % n_cores) if rr % 2 == 0 else (n_cores - 1 - (r % n_cores))
        bin_core[b] = c
        bin_tile[b] = tile_ctr[c]
        tile_ctr[c] += 1

    K = int(np.ceil(bin_sums.max() / 128))

    eb = assign[dst_all]
    core_e = bin_core[eb].astype(np.int64)
    tile_e = bin_tile[eb].astype(np.int64)
    dstloc = slot[dst_all].astype(np.float32)

    key = core_e * tiles + tile_e
    counts = np.bincount(key, minlength=n_bins)
    order = np.argsort(key, kind="stable")
    key_s = key[order]
    starts = np.concatenate([[0], np.cumsum(counts)])[:-1]
    rank = np.arange(e_tot, dtype=np.int64) - starts[key_s]
    eslot = (key_s % tiles) * (K * 128) + rank
    core_s = (key_s // tiles).astype(np.int64)

    # first-slot-per-dst marker (for folding b2 into one message per node)
    ds = dst_all[order]
    first = np.zeros(e_tot, dtype=bool)
    seen = np.zeros(n_nodes, dtype=bool)
    # edges are grouped by (core,tile) and stable-ordered; mark first occurrence
    idx_first = np.unique(ds, return_index=True)[1]
    first[idx_first] = True

    return {
        "K": K,
        "src": src_all[order],
        "dst": ds,
        "norm": norm[order],
        "dstloc": dstloc[order],
        "slot": eslot,
        "core": core_s,
        "first": first,
        "bin_core": bin_core,
        "bin_tile": bin_tile,
        "node_bin": assign,
        "node_slot": slot,
    }


def edge_payload(shard, table, c, *, tiles=TILES, bias=None):
    """[128, T*K*D] per-core array: slot (t,g,p) holds table[src]*norm (+bias
    on the first slot of each dst segment)."""
    K = shard["K"]
    D = table.shape[1]
    size = tiles * K * 128
    m = shard["core"] == c
    arr = np.zeros((size, D), dtype=np.float32)
    vals = table[shard["src"][m]] * shard["norm"][m][:, None]
    if bias is not None:
        fm = shard["first"][m]
        vals[fm] += bias.reshape(1, -1)
    arr[shard["slot"][m]] = vals
    out = arr.reshape(tiles, K, 128, D).transpose(2, 0, 1, 3)
    return np.ascontiguousarray(out.reshape(128, tiles * K * D)).astype(
        _edge_np())


def dstloc_payload(shard, c, *, tiles=TILES):
    K = shard["K"]
    size = tiles * K * 128
    m = shard["core"] == c
    arr = np.full(size, 255.0, dtype=np.float32)  # pad: matches no iota col
    arr[shard["slot"][m]] = shard["dstloc"][m]
    out = arr.reshape(tiles, K, 128).transpose(2, 0, 1)
    return np.ascontiguousarray(out.reshape(128, tiles * K))


def gather_rows(shard, results, name, *, n_nodes=N_NODES):
    """Undo the node permutation: rows for node n live at
    results[bin_core[bin]][name][bin_tile[bin]*128 + slot]."""
    b = shard["node_bin"]
    rows = shard["bin_tile"][b].astype(np.int64) * 128 + shard["node_slot"]
    cores = shard["bin_core"][b]
    dim = results[0][name].shape[1]
    out = np.empty((n_nodes, dim), dtype=np.float32)
    for c in range(len(results)):
        m = cores == c
        out[m] = results[c][name][rows[m]]
    return out


def _log(msg):
    if os.environ.get("BASSGCN_VERBOSE", "0") == "1":
        import time
        print(f"[bassgcn +{time.time() % 100000:.1f}] {msg}", flush=True)


def kernel(x, edge_index, W1, b1, W2, b2):
    global LAST_RESULTS
    from concourse.bass_utils import run_bass_kernel_spmd

    trace = os.environ.get("BASSGCN_TRACE", "0") == "1"
    if trace:
        _install_axon_ntff_hook()

    x = np.ascontiguousarray(np.asarray(x, dtype=np.float32))
    W1 = np.ascontiguousarray(np.asarray(W1, dtype=np.float32))
    W2 = np.ascontiguousarray(np.asarray(W2, dtype=np.float32))
    b1 = np.asarray(b1, dtype=np.float32).reshape(-1, 1)
    b2 = np.asarray(b2, dtype=np.float32).reshape(-1)

    _log("preprocess start")
    shard = shard_edges(edge_index)
    K = shard["K"]
    _log(f"shard done K={K}")
    iota = np.broadcast_to(np.arange(128, dtype=np.float32),
                           (128, 128)).astype(_edge_np()).copy()
    ident = np.eye(OUT_DIM, dtype=np.float32)
    dl = [dstloc_payload(shard, c) for c in range(N_CORES)]

    nc1 = build_phase1(K)
    _log("phase1 built+compiled")
    in_maps1 = []
    for c in range(N_CORES):
        in_maps1.append({
            "xe": edge_payload(shard, x, c),
            "dstloc": dl[c], "W1": W1, "W2": W2, "b1": b1,
            "iota": iota, "ident": ident,
        })
    _log("phase1 payloads ready")
    res1 = run_bass_kernel_spmd(nc1, in_maps1, core_ids=list(range(N_CORES)),
                                trace=trace)
    _log("phase1 ran")

    h2 = gather_rows(shard, [res1.results[c] for c in range(N_CORES)], "h2")

    nc2 = build_phase2(K)
    _log("phase2 built+compiled")
    in_maps2 = []
    for c in range(N_CORES):
        in_maps2.append({
            "he": edge_payload(shard, h2, c, bias=b2),
            "dstloc": dl[c], "iota": iota,
        })
    _log("phase2 payloads ready")
    res2 = run_bass_kernel_spmd(nc2, in_maps2, core_ids=list(range(N_CORES)),
                                trace=trace)
    _log("phase2 ran")
    LAST_RESULTS = [res1, res2]

    out = gather_rows(shard, [res2.results[c] for c in range(N_CORES)], "out")
    return out.astype(np.float32)


# revision 8
# speedup vs baseline: 2.3531x; 2.3531x over previous
"""GCN (2-layer, PyG GCNConv-style) on 8 Trainium2 NeuronCores.

Sharding: destination nodes are sorted by degree and packed into tiles of
128 nodes each, one node per partition row; tiles are dealt round-robin to
the 8 cores so all cores see the same per-tile chunk-count sequence (the
group max), keeping the program SPMD-uniform. Because every node owns its
partition row, the segment-sum over a node's edges is just an accumulation
of 128-edge chunks: chunk g holds the g-th edge message of every node in
the tile, and the aggregation is a chain of PSUM-accumulating matmuls
against a constant identity matrix (no per-chunk index work at all).
Degree-sorted packing makes the padding (chunks = max degree in tile) ~1%.

Edges (incl. self-loops) carry the symmetric D^-1/2 A D^-1/2 norm. The host
lays the scaled fp16 edge messages out contiguously per (core, tile, chunk),
so every device access is a sequential stream - the static gather pattern is
folded into the input sharding. By linearity segsum(h[src]*norm) =
segsum(x[src]*norm) @ W1, so the dense transforms run once per node after
aggregation. Phase 1 computes h2 = relu(agg1@W1 + b1)@W2 per node shard; the
host then routes the h2 edge messages (the halo exchange) and phase 2
segment-sums them (b2 is folded into the first message slot of each node).
"""

import math
import os
import sys
import types

sys.path.insert(0, "/opt/trn_rl_repo")

import numpy as np

N_NODES = 100000
IN_DIM = 128
HID_DIM = 128
OUT_DIM = 64
N_CORES = 8

LAST_RESULTS = []  # BassKernelResults of the most recent kernel() call


def _install_axon_ntff_hook():
    """Register the NTFF profiling hook that the stub antenv package lacks."""
    try:
        import antenv
        if getattr(antenv, "axon_hooks", None) is not None:
            return
        hooks_mod = types.ModuleType("antenv.axon_hooks")
        _hook = [None]
        hooks_mod.set_axon_ntff_profile_hook = lambda h: _hook.__setitem__(0, h)
        hooks_mod.get_axon_ntff_profile_hook = lambda: _hook[0]
        sys.modules["antenv.axon_hooks"] = hooks_mod
        antenv.axon_hooks = hooks_mod
        from trn_agent_boot.trn_boot import _ntff_profile_via_ctypes
        hooks_mod.set_axon_ntff_profile_hook(
            _ntff_profile_via_ctypes("/opt/axon/libaxon_pjrt.so")
        )
    except Exception:
        pass


def _edge_dt():
    import concourse.mybir as mybir
    d = os.environ.get("BASSGCN_EDGE_DT", "fp16")
    return {"bf16": mybir.dt.bfloat16, "fp16": mybir.dt.float16,
            "fp32": mybir.dt.float32}[d]


def _edge_np():
    d = os.environ.get("BASSGCN_EDGE_DT", "fp16")
    if d == "fp32":
        return np.float32
    if d == "fp16":
        return np.float16
    import ml_dtypes
    return ml_dtypes.bfloat16


def build_phase1(Ks, *, in_dim=IN_DIM, hid_dim=HID_DIM, out_dim=OUT_DIM,
                 n_cores=N_CORES):
    """x-messages (pre-gathered, pre-scaled) -> h2 = relu(agg@W1+b1)@W2."""
    import concourse.bacc as bacc
    import concourse.mybir as mybir
    import concourse.tile as tile

    f32 = mybir.dt.float32
    edt = _edge_dt()
    chk_total = int(sum(Ks))
    tiles = len(Ks)
    kmax = int(max(Ks))
    off = np.concatenate([[0], np.cumsum(Ks)]).astype(int)

    nc = bacc.Bacc("TRN2", target_bir_lowering=False, debug=False,
                   num_devices=n_cores)
    xe = nc.dram_tensor("xe", [128, chk_total * in_dim], edt,
                        kind="ExternalInput")
    w1i = nc.dram_tensor("W1", [in_dim, hid_dim], f32, kind="ExternalInput")
    w2i = nc.dram_tensor("W2", [hid_dim, out_dim], f32, kind="ExternalInput")
    b1i = nc.dram_tensor("b1", [hid_dim, 1], f32, kind="ExternalInput")
    ici = nc.dram_tensor("identc", [128, 128], edt, kind="ExternalInput")
    idi = nc.dram_tensor("ident", [out_dim, out_dim], f32,
                         kind="ExternalInput")
    h2o = nc.dram_tensor("h2", [tiles * 128, out_dim], f32,
                         kind="ExternalOutput")

    Relu = mybir.ActivationFunctionType.Relu

    with tile.TileContext(nc) as tc:
        with (
            tc.tile_pool(name="const", bufs=1) as cp,
            tc.tile_pool(name="gath", bufs=3) as gp,
            tc.tile_pool(name="epi", bufs=3) as ep,
            tc.tile_pool(name="psum", bufs=2, space="PSUM") as pp,
        ):
            w1s = cp.tile([in_dim, hid_dim], f32, name="w1s")
            nc.sync.dma_start(out=w1s[:], in_=w1i[:, :])
            w2s = cp.tile([hid_dim, out_dim], f32, name="w2s")
            nc.sync.dma_start(out=w2s[:], in_=w2i[:, :])
            b1s = cp.tile([hid_dim, 1], f32, name="b1s")
            nc.sync.dma_start(out=b1s[:], in_=b1i[:, :])
            ics = cp.tile([128, 128], edt, name="ics")
            nc.sync.dma_start(out=ics[:], in_=ici[:, :])
            idents = cp.tile([out_dim, out_dim], f32, name="idents")
            nc.sync.dma_start(out=idents[:], in_=idi[:, :])

            for t in range(tiles):
                K = int(Ks[t])
                if K == 0:
                    continue
                o0 = int(off[t])
                xg = gp.tile([128, kmax * in_dim], edt, name="xg", tag="xg")
                nc.sync.dma_start(
                    out=xg[:, :K * in_dim],
                    in_=xe[:, o0 * in_dim:(o0 + K) * in_dim])
                pA = pp.tile([128, 128], f32, name="pA", tag="acc")
                for g in range(K):
                    # aggxT[k, d] += xg_g[d, k]  (identity-selected transpose)
                    nc.tensor.matmul(
                        out=pA[:], lhsT=xg[:, g * in_dim:(g + 1) * in_dim],
                        rhs=ics[:], start=(g == 0), stop=(g == K - 1))
                # h1T = relu(W1^T @ aggxT + b1)
                aggxT = ep.tile([128, 128], f32, name="aggxT", tag="aggxT")
                nc.scalar.copy(out=aggxT[:], in_=pA[:])
                pB = pp.tile([128, 128], f32, name="pB", tag="fin")
                nc.tensor.matmul(out=pB[:], lhsT=w1s[:], rhs=aggxT[:],
                                 start=True, stop=True)
                h1T = ep.tile([128, 128], f32, name="h1T", tag="h1T")
                nc.scalar.activation(out=h1T[:], in_=pB[:], func=Relu,
                                     bias=b1s[:, 0:1], scale=1.0)
                # h2T = W2^T @ h1T (b2 is folded into the phase-2 messages)
                pC = pp.tile([out_dim, 128], f32, name="pC", tag="fin")
                nc.tensor.matmul(out=pC[:], lhsT=w2s[:], rhs=h1T[:],
                                 start=True, stop=True)
                h2T = ep.tile([out_dim, 128], f32, name="h2T", tag="h2T")
                nc.scalar.copy(out=h2T[:], in_=pC[:])
                pD = pp.tile([128, out_dim], f32, name="pD", tag="fin")
                nc.tensor.transpose(out=pD[:], in_=h2T[:], identity=idents[:])
                h2t = ep.tile([128, out_dim], f32, name="h2t", tag="h2t")
                nc.scalar.copy(out=h2t[:], in_=pD[:])
                nc.sync.dma_start(out=h2o[t * 128:(t + 1) * 128, :],
                                  in_=h2t[:])
    nc.compile()
    return nc


def build_phase2(Ks, *, out_dim=OUT_DIM, n_cores=N_CORES):
    """h2-messages (pre-gathered, pre-scaled, +b2 folded) -> out = segsum."""
    import concourse.bacc as bacc
    import concourse.mybir as mybir
    import concourse.tile as tile

    f32 = mybir.dt.float32
    edt = _edge_dt()
    chk_total = int(sum(Ks))
    tiles = len(Ks)
    kmax = int(max(Ks))
    off = np.concatenate([[0], np.cumsum(Ks)]).astype(int)

    nc = bacc.Bacc("TRN2", target_bir_lowering=False, debug=False,
                   num_devices=n_cores)
    he = nc.dram_tensor("he", [128, chk_total * out_dim], edt,
                        kind="ExternalInput")
    ici = nc.dram_tensor("identc", [128, 128], edt, kind="ExternalInput")
    out_t = nc.dram_tensor("out", [tiles * 128, out_dim], f32,
                           kind="ExternalOutput")

    with tile.TileContext(nc) as tc:
        with (
            tc.tile_pool(name="const", bufs=1) as cp,
            tc.tile_pool(name="gath", bufs=3) as gp,
            tc.tile_pool(name="epi", bufs=3) as ep,
            tc.tile_pool(name="psum", bufs=2, space="PSUM") as pp,
        ):
            ics = cp.tile([128, 128], edt, name="ics")
            nc.sync.dma_start(out=ics[:], in_=ici[:, :])

            for t in range(tiles):
                K = int(Ks[t])
                if K == 0:
                    continue
                o0 = int(off[t])
                hg = gp.tile([128, kmax * out_dim], edt, name="hg", tag="hg")
                nc.sync.dma_start(
                    out=hg[:, :K * out_dim],
                    in_=he[:, o0 * out_dim:(o0 + K) * out_dim])
                pE = pp.tile([128, out_dim], f32, name="pE", tag="acc")
                for g in range(K):
                    # out[d, o] += hg_g[d, o]  (identity as resident weights)
                    nc.tensor.matmul(
                        out=pE[:], lhsT=ics[:],
                        rhs=hg[:, g * out_dim:(g + 1) * out_dim],
                        start=(g == 0), stop=(g == K - 1))
                ot = ep.tile([128, out_dim], f32, name="ot", tag="ot")
                nc.scalar.copy(out=ot[:], in_=pE[:])
                nc.sync.dma_start(out=out_t[t * 128:(t + 1) * 128, :],
                                  in_=ot[:])
    nc.compile()
    return nc


def shard_edges(edge_index, *, n_nodes=N_NODES, n_cores=N_CORES):
    """Degree-sorted node packing + per-(core,tile,chunk) edge slotting."""
    ei = np.asarray(edge_index)
    src = ei[0].astype(np.int64)
    dst = ei[1].astype(np.int64)

    loops = np.arange(n_nodes, dtype=np.int64)
    src_all = np.concatenate([src, loops])
    dst_all = np.concatenate([dst, loops])
    e_tot = src_all.shape[0]

    deg = np.bincount(dst_all, minlength=n_nodes).astype(np.int64)
    dis = 1.0 / np.sqrt(deg.astype(np.float64))  # self-loops => deg >= 1
    norm = (dis[src_all] * dis[dst_all]).astype(np.float32)

    n_blocks = math.ceil(n_nodes / 128)
    n_groups = math.ceil(n_blocks / n_cores)

    srt = np.argsort(-deg, kind="stable")  # node ranks by degree desc
    rank = np.empty(n_nodes, np.int64)
    rank[srt] = np.arange(n_nodes)
    node_block = rank // 128
    node_row = rank % 128
    node_core = node_block % n_cores
    node_tile = node_block // n_cores

    # per-tile chunk count = max degree in the tile group (same on all cores)
    Ks = np.zeros(n_groups, np.int64)
    deg_sorted = deg[srt]
    for t in range(n_groups):
        lo = t * n_cores * 128
        if lo < n_nodes:
            Ks[t] = deg_sorted[lo]  # degree-desc => group max is first entry
    off = np.concatenate([[0], np.cumsum(Ks)]).astype(np.int64)
    chk_total = int(off[-1])

    # edge slot: chunk g = rank within dst's edge list, partition = node row
    e_core = node_core[dst_all]
    e_tile = node_tile[dst_all]
    e_row = node_row[dst_all]
    order = np.argsort(dst_all, kind="stable")
    counts = np.bincount(dst_all, minlength=n_nodes)
    starts = np.concatenate([[0], np.cumsum(counts)])[:-1]
    g_of = np.empty(e_tot, np.int64)
    g_of[order] = np.arange(e_tot) - starts[dst_all[order]]

    slot = (off[e_tile] + g_of) * 128 + e_row  # within the core's array
    first = g_of == 0

    return {
        "Ks": Ks,
        "chk_total": chk_total,
        "src": src_all,
        "norm": norm,
        "slot": slot,
        "core": e_core,
        "first": first,
        "node_core": node_core,
        "node_tile": node_tile,
        "node_row": node_row,
    }


def edge_payload(shard, table, c, *, bias=None):
    """[128, chk_total*D]: slot (t,g,p) holds table[src]*norm (+bias on the
    first slot of each dst segment)."""
    D = table.shape[1]
    chk_total = shard["chk_total"]
    m = shard["core"] == c
    arr = np.zeros((chk_total * 128, D), dtype=np.float32)
    vals = table[shard["src"][m]] * shard["norm"][m][:, None]
    if bias is not None:
        fm = shard["first"][m]
        vals[fm] += bias.reshape(1, -1)
    arr[shard["slot"][m]] = vals
    out = arr.reshape(chk_total, 128, D).transpose(1, 0, 2)
    return np.ascontiguousarray(out.reshape(128, chk_total * D)).astype(
        _edge_np())


def gather_rows(shard, results, name, *, n_nodes=N_NODES):
    """Undo the node permutation."""
    rows = shard["node_tile"] * 128 + shard["node_row"]
    cores = shard["node_core"]
    dim = results[0][name].shape[1]
    out = np.empty((n_nodes, dim), dtype=np.float32)
    for c in range(len(results)):
        m = cores == c
        out[m] = results[c][name][rows[m]]
    return out


def _log(msg):
    if os.environ.get("BASSGCN_VERBOSE", "0") == "1":
        import time
        print(f"[bassgcn +{time.time() % 100000:.1f}] {msg}", flush=True)


def kernel(x, edge_index, W1, b1, W2, b2):
    global LAST_RESULTS
    from concourse.bass_utils import run_bass_kernel_spmd

    trace = os.environ.get("BASSGCN_TRACE", "0") == "1"
    if trace:
        _install_axon_ntff_hook()

    x = np.ascontiguousarray(np.asarray(x, dtype=np.float32))
    W1 = np.ascontiguousarray(np.asarray(W1, dtype=np.float32))
    W2 = np.ascontiguousarray(np.asarray(W2, dtype=np.float32))
    b1 = np.asarray(b1, dtype=np.float32).reshape(-1, 1)
    b2 = np.asarray(b2, dtype=np.float32).reshape(-1)

    _log("preprocess start")
    shard = shard_edges(edge_index)
    Ks = shard["Ks"]
    _log(f"shard done chunks={shard['chk_total']} kmax={int(Ks.max())}")
    identc = np.eye(128, dtype=np.float32).astype(_edge_np())
    ident = np.eye(OUT_DIM, dtype=np.float32)

    nc1 = build_phase1(Ks)
    _log("phase1 built+compiled")
    in_maps1 = []
    for c in range(N_CORES):
        in_maps1.append({
            "xe": edge_payload(shard, x, c),
            "W1": W1, "W2": W2, "b1": b1, "identc": identc, "ident": ident,
        })
    _log("phase1 payloads ready")
    res1 = run_bass_kernel_spmd(nc1, in_maps1, core_ids=list(range(N_CORES)),
                                trace=trace)
    _log("phase1 ran")

    h2 = gather_rows(shard, [res1.results[c] for c in range(N_CORES)], "h2")

    nc2 = build_phase2(Ks)
    _log("phase2 built+compiled")
    in_maps2 = []
    for c in range(N_CORES):
        in_maps2.append({
            "he": edge_payload(shard, h2, c, bias=b2),
            "identc": identc,
        })
    _log("phase2 payloads ready")
    res2 = run_bass_kernel_spmd(nc2, in_maps2, core_ids=list(range(N_CORES)),
                                trace=trace)
    _log("phase2 ran")
    LAST_RESULTS = [res1, res2]

    out = gather_rows(shard, [res2.results[c] for c in range(N_CORES)], "out")
    return out.astype(np.float32)


# revision 9
# speedup vs baseline: 2.6387x; 1.1214x over previous
"""GCN (2-layer, PyG GCNConv-style) on 8 Trainium2 NeuronCores.

Sharding: destination nodes are sorted by degree and packed into tiles of
128 nodes each, one node per partition row; tiles are dealt round-robin to
the 8 cores so all cores see the same per-tile chunk-count sequence (the
group max), keeping the program SPMD-uniform. Because every node owns its
partition row, the segment-sum over a node's edges is just an accumulation
of 128-edge chunks: chunk g holds the g-th edge message of every node in
the tile, and the aggregation is a chain of PSUM-accumulating matmuls
against a constant identity matrix (no per-chunk index work at all).
Degree-sorted packing makes the padding (chunks = max degree in tile) ~1%.

Edges (incl. self-loops) carry the symmetric D^-1/2 A D^-1/2 norm. The host
lays the scaled fp16 edge messages out contiguously per (core, tile, chunk),
so every device access is a sequential stream - the static gather pattern is
folded into the input sharding. By linearity segsum(h[src]*norm) =
segsum(x[src]*norm) @ W1, so the dense transforms run once per node after
aggregation. Phase 1 computes h2 = relu(agg1@W1 + b1)@W2 per node shard; the
host then routes the h2 edge messages (the halo exchange) and phase 2
segment-sums them (b2 is folded into the first message slot of each node).
"""

import math
import os
import sys
import types

sys.path.insert(0, "/opt/trn_rl_repo")

import numpy as np

N_NODES = 100000
IN_DIM = 128
HID_DIM = 128
OUT_DIM = 64
N_CORES = 8

LAST_RESULTS = []  # BassKernelResults of the most recent kernel() call


def _install_axon_ntff_hook():
    """Register the NTFF profiling hook that the stub antenv package lacks."""
    try:
        import antenv
        if getattr(antenv, "axon_hooks", None) is not None:
            return
        hooks_mod = types.ModuleType("antenv.axon_hooks")
        _hook = [None]
        hooks_mod.set_axon_ntff_profile_hook = lambda h: _hook.__setitem__(0, h)
        hooks_mod.get_axon_ntff_profile_hook = lambda: _hook[0]
        sys.modules["antenv.axon_hooks"] = hooks_mod
        antenv.axon_hooks = hooks_mod
        from trn_agent_boot.trn_boot import _ntff_profile_via_ctypes
        hooks_mod.set_axon_ntff_profile_hook(
            _ntff_profile_via_ctypes("/opt/axon/libaxon_pjrt.so")
        )
    except Exception:
        pass


def _edge_dt():
    import concourse.mybir as mybir
    d = os.environ.get("BASSGCN_EDGE_DT", "fp16")
    return {"bf16": mybir.dt.bfloat16, "fp16": mybir.dt.float16,
            "fp32": mybir.dt.float32}[d]


def _edge_np():
    d = os.environ.get("BASSGCN_EDGE_DT", "fp16")
    if d == "fp32":
        return np.float32
    if d == "fp16":
        return np.float16
    import ml_dtypes
    return ml_dtypes.bfloat16


def build_phase1(Ks, *, in_dim=IN_DIM, hid_dim=HID_DIM, out_dim=OUT_DIM,
                 n_cores=N_CORES):
    """x-messages (pre-gathered, pre-scaled) -> h2 = relu(agg@W1+b1)@W2."""
    import concourse.bacc as bacc
    import concourse.mybir as mybir
    import concourse.tile as tile

    f32 = mybir.dt.float32
    edt = _edge_dt()
    chk_total = int(sum(Ks))
    tiles = len(Ks)
    kmax = int(max(Ks))
    off = np.concatenate([[0], np.cumsum(Ks)]).astype(int)

    nc = bacc.Bacc("TRN2", target_bir_lowering=False, debug=False,
                   num_devices=n_cores)
    xe = nc.dram_tensor("xe", [128, chk_total * in_dim], edt,
                        kind="ExternalInput")
    w1i = nc.dram_tensor("W1", [in_dim, hid_dim], f32, kind="ExternalInput")
    w2i = nc.dram_tensor("W2", [hid_dim, out_dim], f32, kind="ExternalInput")
    b1i = nc.dram_tensor("b1", [hid_dim, 1], f32, kind="ExternalInput")
    ici = nc.dram_tensor("identc", [128, 128], edt, kind="ExternalInput")
    idi = nc.dram_tensor("ident", [out_dim, out_dim], f32,
                         kind="ExternalInput")
    h2o = nc.dram_tensor("h2", [tiles * 128, out_dim], f32,
                         kind="ExternalOutput")

    Relu = mybir.ActivationFunctionType.Relu

    with tile.TileContext(nc) as tc:
        with (
            tc.tile_pool(name="const", bufs=1) as cp,
            tc.tile_pool(name="gath", bufs=3) as gp,
            tc.tile_pool(name="epi", bufs=3) as ep,
            tc.tile_pool(name="psum", bufs=2, space="PSUM") as pp,
        ):
            w1s = cp.tile([in_dim, hid_dim], f32, name="w1s")
            nc.sync.dma_start(out=w1s[:], in_=w1i[:, :])
            w2s = cp.tile([hid_dim, out_dim], f32, name="w2s")
            nc.sync.dma_start(out=w2s[:], in_=w2i[:, :])
            b1s = cp.tile([hid_dim, 1], f32, name="b1s")
            nc.sync.dma_start(out=b1s[:], in_=b1i[:, :])
            ics = cp.tile([128, 128], edt, name="ics")
            nc.sync.dma_start(out=ics[:], in_=ici[:, :])
            idents = cp.tile([out_dim, out_dim], f32, name="idents")
            nc.sync.dma_start(out=idents[:], in_=idi[:, :])

            for t in range(tiles):
                K = int(Ks[t])
                if K == 0:
                    continue
                o0 = int(off[t])
                xg = gp.tile([128, kmax * in_dim], edt, name="xg", tag="xg")
                deng = nc.scalar if t % 2 == 0 else nc.sync
                deng.dma_start(
                    out=xg[:, :K * in_dim],
                    in_=xe[:, o0 * in_dim:(o0 + K) * in_dim])
                pA = pp.tile([128, 128], f32, name="pA", tag="acc")
                for g in range(K):
                    # aggxT[k, d] += xg_g[d, k]  (identity-selected transpose)
                    nc.tensor.matmul(
                        out=pA[:], lhsT=xg[:, g * in_dim:(g + 1) * in_dim],
                        rhs=ics[:], start=(g == 0), stop=(g == K - 1))
                # h1T = relu(W1^T @ aggxT + b1)
                aggxT = ep.tile([128, 128], f32, name="aggxT", tag="aggxT")
                nc.vector.tensor_copy(out=aggxT[:], in_=pA[:])
                pB = pp.tile([128, 128], f32, name="pB", tag="finB")
                nc.tensor.matmul(out=pB[:], lhsT=w1s[:], rhs=aggxT[:],
                                 start=True, stop=True)
                h1T = ep.tile([128, 128], f32, name="h1T", tag="h1T")
                nc.scalar.activation(out=h1T[:], in_=pB[:], func=Relu,
                                     bias=b1s[:, 0:1], scale=1.0)
                # h2T = W2^T @ h1T (b2 is folded into the phase-2 messages)
                pC = pp.tile([out_dim, 128], f32, name="pC", tag="finC")
                nc.tensor.matmul(out=pC[:], lhsT=w2s[:], rhs=h1T[:],
                                 start=True, stop=True)
                h2T = ep.tile([out_dim, 128], f32, name="h2T", tag="h2T")
                nc.scalar.copy(out=h2T[:], in_=pC[:])
                pD = pp.tile([128, out_dim], f32, name="pD", tag="finD")
                nc.tensor.transpose(out=pD[:], in_=h2T[:], identity=idents[:])
                h2t = ep.tile([128, out_dim], f32, name="h2t", tag="h2t")
                nc.vector.tensor_copy(out=h2t[:], in_=pD[:])
                oeng = nc.sync if t % 2 == 0 else nc.scalar
                oeng.dma_start(out=h2o[t * 128:(t + 1) * 128, :],
                               in_=h2t[:])
    nc.compile()
    return nc


def build_phase2(Ks, *, out_dim=OUT_DIM, n_cores=N_CORES):
    """h2-messages (pre-gathered, pre-scaled, +b2 folded) -> out = segsum."""
    import concourse.bacc as bacc
    import concourse.mybir as mybir
    import concourse.tile as tile

    f32 = mybir.dt.float32
    edt = _edge_dt()
    chk_total = int(sum(Ks))
    tiles = len(Ks)
    kmax = int(max(Ks))
    off = np.concatenate([[0], np.cumsum(Ks)]).astype(int)

    nc = bacc.Bacc("TRN2", target_bir_lowering=False, debug=False,
                   num_devices=n_cores)
    he = nc.dram_tensor("he", [128, chk_total * out_dim], edt,
                        kind="ExternalInput")
    ici = nc.dram_tensor("identc", [128, 128], edt, kind="ExternalInput")
    out_t = nc.dram_tensor("out", [tiles * 128, out_dim], f32,
                           kind="ExternalOutput")

    with tile.TileContext(nc) as tc:
        with (
            tc.tile_pool(name="const", bufs=1) as cp,
            tc.tile_pool(name="gath", bufs=3) as gp,
            tc.tile_pool(name="epi", bufs=3) as ep,
            tc.tile_pool(name="psum", bufs=2, space="PSUM") as pp,
        ):
            ics = cp.tile([128, 128], edt, name="ics")
            nc.sync.dma_start(out=ics[:], in_=ici[:, :])

            for t in range(tiles):
                K = int(Ks[t])
                if K == 0:
                    continue
                o0 = int(off[t])
                hg = gp.tile([128, kmax * out_dim], edt, name="hg", tag="hg")
                deng = nc.scalar if t % 2 == 0 else nc.sync
                deng.dma_start(
                    out=hg[:, :K * out_dim],
                    in_=he[:, o0 * out_dim:(o0 + K) * out_dim])
                pE = pp.tile([128, out_dim], f32, name="pE", tag="acc")
                for g in range(K):
                    # out[d, o] += hg_g[d, o]  (identity as resident weights)
                    nc.tensor.matmul(
                        out=pE[:], lhsT=ics[:],
                        rhs=hg[:, g * out_dim:(g + 1) * out_dim],
                        start=(g == 0), stop=(g == K - 1))
                ot = ep.tile([128, out_dim], f32, name="ot", tag="ot")
                nc.vector.tensor_copy(out=ot[:], in_=pE[:])
                oeng = nc.sync if t % 2 == 0 else nc.scalar
                oeng.dma_start(out=out_t[t * 128:(t + 1) * 128, :],
                               in_=ot[:])
    nc.compile()
    return nc


def shard_edges(edge_index, *, n_nodes=N_NODES, n_cores=N_CORES):
    """Degree-sorted node packing + per-(core,tile,chunk) edge slotting."""
    ei = np.asarray(edge_index)
    src = ei[0].astype(np.int64)
    dst = ei[1].astype(np.int64)

    loops = np.arange(n_nodes, dtype=np.int64)
    src_all = np.concatenate([src, loops])
    dst_all = np.concatenate([dst, loops])
    e_tot = src_all.shape[0]

    deg = np.bincount(dst_all, minlength=n_nodes).astype(np.int64)
    dis = 1.0 / np.sqrt(deg.astype(np.float64))  # self-loops => deg >= 1
    norm = (dis[src_all] * dis[dst_all]).astype(np.float32)

    n_blocks = math.ceil(n_nodes / 128)
    n_groups = math.ceil(n_blocks / n_cores)

    srt = np.argsort(-deg, kind="stable")  # node ranks by degree desc
    rank = np.empty(n_nodes, np.int64)
    rank[srt] = np.arange(n_nodes)
    node_block = rank // 128
    node_row = rank % 128
    node_core = node_block % n_cores
    node_tile = node_block // n_cores

    # per-tile chunk count = max degree in the tile group (same on all cores)
    Ks = np.zeros(n_groups, np.int64)
    deg_sorted = deg[srt]
    for t in range(n_groups):
        lo = t * n_cores * 128
        if lo < n_nodes:
            Ks[t] = deg_sorted[lo]  # degree-desc => group max is first entry
    off = np.concatenate([[0], np.cumsum(Ks)]).astype(np.int64)
    chk_total = int(off[-1])

    # edge slot: chunk g = rank within dst's edge list, partition = node row
    e_core = node_core[dst_all]
    e_tile = node_tile[dst_all]
    e_row = node_row[dst_all]
    order = np.argsort(dst_all, kind="stable")
    counts = np.bincount(dst_all, minlength=n_nodes)
    starts = np.concatenate([[0], np.cumsum(counts)])[:-1]
    g_of = np.empty(e_tot, np.int64)
    g_of[order] = np.arange(e_tot) - starts[dst_all[order]]

    slot = (off[e_tile] + g_of) * 128 + e_row  # within the core's array
    first = g_of == 0

    return {
        "Ks": Ks,
        "chk_total": chk_total,
        "src": src_all,
        "norm": norm,
        "slot": slot,
        "core": e_core,
        "first": first,
        "node_core": node_core,
        "node_tile": node_tile,
        "node_row": node_row,
    }


def edge_payload(shard, table, c, *, bias=None):
    """[128, chk_total*D]: slot (t,g,p) holds table[src]*norm (+bias on the
    first slot of each dst segment)."""
    D = table.shape[1]
    chk_total = shard["chk_total"]
    m = shard["core"] == c
    arr = np.zeros((chk_total * 128, D), dtype=np.float32)
    vals = table[shard["src"][m]] * shard["norm"][m][:, None]
    if bias is not None:
        fm = shard["first"][m]
        vals[fm] += bias.reshape(1, -1)
    arr[shard["slot"][m]] = vals
    out = arr.reshape(chk_total, 128, D).transpose(1, 0, 2)
    return np.ascontiguousarray(out.reshape(128, chk_total * D)).astype(
        _edge_np())


def gather_rows(shard, results, name, *, n_nodes=N_NODES):
    """Undo the node permutation."""
    rows = shard["node_tile"] * 128 + shard["node_row"]
    cores = shard["node_core"]
    dim = results[0][name].shape[1]
    out = np.empty((n_nodes, dim), dtype=np.float32)
    for c in range(len(results)):
        m = cores == c
        out[m] = results[c][name][rows[m]]
    return out


def _log(msg):
    if os.environ.get("BASSGCN_VERBOSE", "0") == "1":
        import time
        print(f"[bassgcn +{time.time() % 100000:.1f}] {msg}", flush=True)


def kernel(x, edge_index, W1, b1, W2, b2):
    global LAST_RESULTS
    from concourse.bass_utils import run_bass_kernel_spmd

    trace = os.environ.get("BASSGCN_TRACE", "0") == "1"
    if trace:
        _install_axon_ntff_hook()

    x = np.ascontiguousarray(np.asarray(x, dtype=np.float32))
    W1 = np.ascontiguousarray(np.asarray(W1, dtype=np.float32))
    W2 = np.ascontiguousarray(np.asarray(W2, dtype=np.float32))
    b1 = np.asarray(b1, dtype=np.float32).reshape(-1, 1)
    b2 = np.asarray(b2, dtype=np.float32).reshape(-1)

    _log("preprocess start")
    shard = shard_edges(edge_index)
    Ks = shard["Ks"]
    _log(f"shard done chunks={shard['chk_total']} kmax={int(Ks.max())}")
    identc = np.eye(128, dtype=np.float32).astype(_edge_np())
    ident = np.eye(OUT_DIM, dtype=np.float32)

    nc1 = build_phase1(Ks)
    _log("phase1 built+compiled")
    in_maps1 = []
    for c in range(N_CORES):
        in_maps1.append({
            "xe": edge_payload(shard, x, c),
            "W1": W1, "W2": W2, "b1": b1, "identc": identc, "ident": ident,
        })
    _log("phase1 payloads ready")
    res1 = run_bass_kernel_spmd(nc1, in_maps1, core_ids=list(range(N_CORES)),
                                trace=trace)
    _log("phase1 ran")

    h2 = gather_rows(shard, [res1.results[c] for c in range(N_CORES)], "h2")

    nc2 = build_phase2(Ks)
    _log("phase2 built+compiled")
    in_maps2 = []
    for c in range(N_CORES):
        in_maps2.append({
            "he": edge_payload(shard, h2, c, bias=b2),
            "identc": identc,
        })
    _log("phase2 payloads ready")
    res2 = run_bass_kernel_spmd(nc2, in_maps2, core_ids=list(range(N_CORES)),
                                trace=trace)
    _log("phase2 ran")
    LAST_RESULTS = [res1, res2]

    out = gather_rows(shard, [res2.results[c] for c in range(N_CORES)], "out")
    return out.astype(np.float32)


# revision 12
# speedup vs baseline: 3.0650x; 1.1616x over previous
"""GCN (2-layer, PyG GCNConv-style) on 8 Trainium2 NeuronCores.

Sharding: destination nodes are sorted by degree and packed into tiles of
128 nodes each, one node per partition row; tiles are dealt round-robin to
the 8 cores so all cores see the same per-tile chunk-count sequence (the
group max), keeping the program SPMD-uniform. Because every node owns its
partition row, the segment-sum over a node's edges is just an accumulation
of 128-edge chunks: chunk g holds the g-th edge message of every node in
the tile, and the aggregation is a chain of PSUM-accumulating matmuls
against a constant identity matrix (no per-chunk index work at all).
Degree-sorted packing makes the padding (chunks = max degree in tile) ~1%.

Edges (incl. self-loops) carry the symmetric D^-1/2 A D^-1/2 norm. The host
lays the scaled fp16 edge messages out contiguously per (core, tile, chunk),
so every device access is a sequential stream - the static gather pattern is
folded into the input sharding. By linearity segsum(h[src]*norm) =
segsum(x[src]*norm) @ W1, so the dense transforms run once per node after
aggregation. Phase 1 computes h2 = relu(agg1@W1 + b1)@W2 per node shard; the
host then routes the h2 edge messages (the halo exchange) and phase 2
segment-sums them (b2 is folded into the first message slot of each node).
"""

import math
import os
import sys
import types

sys.path.insert(0, "/opt/trn_rl_repo")

import numpy as np

N_NODES = 100000
IN_DIM = 128
HID_DIM = 128
OUT_DIM = 64
N_CORES = 8

LAST_RESULTS = []  # BassKernelResults of the most recent kernel() call


def _install_axon_ntff_hook():
    """Register the NTFF profiling hook that the stub antenv package lacks."""
    try:
        import antenv
        if getattr(antenv, "axon_hooks", None) is not None:
            return
        hooks_mod = types.ModuleType("antenv.axon_hooks")
        _hook = [None]
        hooks_mod.set_axon_ntff_profile_hook = lambda h: _hook.__setitem__(0, h)
        hooks_mod.get_axon_ntff_profile_hook = lambda: _hook[0]
        sys.modules["antenv.axon_hooks"] = hooks_mod
        antenv.axon_hooks = hooks_mod
        from trn_agent_boot.trn_boot import _ntff_profile_via_ctypes
        hooks_mod.set_axon_ntff_profile_hook(
            _ntff_profile_via_ctypes("/opt/axon/libaxon_pjrt.so")
        )
    except Exception:
        pass


def _edge_dt():
    import concourse.mybir as mybir
    d = os.environ.get("BASSGCN_EDGE_DT", "fp16")
    return {"bf16": mybir.dt.bfloat16, "fp16": mybir.dt.float16,
            "fp32": mybir.dt.float32}[d]


def _edge_np():
    d = os.environ.get("BASSGCN_EDGE_DT", "fp16")
    if d == "fp32":
        return np.float32
    if d == "fp16":
        return np.float16
    import ml_dtypes
    return ml_dtypes.bfloat16


def build_phase1(Ks, *, in_dim=IN_DIM, hid_dim=HID_DIM, out_dim=OUT_DIM,
                 n_cores=N_CORES):
    """x-messages (transposed, pre-scaled) -> h2T = (relu(W1^T@aggxT+b1))^T@W2.

    The payload chunk g of tile t is stored as [k, d]; the W1 transform is
    folded into the accumulation: h1T = sum_g W1^T @ xgT_g, with W1 resident
    in the PE array. h2 is written out transposed; the host undoes it.
    """
    import concourse.bacc as bacc
    import concourse.mybir as mybir
    import concourse.tile as tile

    f32 = mybir.dt.float32
    edt = _edge_dt()
    chk_total = int(sum(Ks))
    tiles = len(Ks)
    kmax = int(max(Ks))
    off = np.concatenate([[0], np.cumsum(Ks)]).astype(int)

    nc = bacc.Bacc("TRN2", target_bir_lowering=False, debug=False,
                   num_devices=n_cores)
    xe = nc.dram_tensor("xe", [128, chk_total * 128], edt,
                        kind="ExternalInput")
    w1i = nc.dram_tensor("W1h", [in_dim, hid_dim], edt, kind="ExternalInput")
    w2i = nc.dram_tensor("W2h", [hid_dim, out_dim], edt, kind="ExternalInput")
    b1i = nc.dram_tensor("b1", [hid_dim, 1], f32, kind="ExternalInput")
    h2o = nc.dram_tensor("h2", [tiles * out_dim, 128], f32,
                         kind="ExternalOutput")

    Relu = mybir.ActivationFunctionType.Relu

    with tile.TileContext(nc) as tc:
        with (
            tc.tile_pool(name="const", bufs=1) as cp,
            tc.tile_pool(name="gath", bufs=4) as gp,
            tc.tile_pool(name="epi", bufs=3) as ep,
            tc.tile_pool(name="psum", bufs=2, space="PSUM") as pp,
        ):
            w1s = cp.tile([in_dim, hid_dim], edt, name="w1s")
            nc.sync.dma_start(out=w1s[:], in_=w1i[:, :])
            w2s = cp.tile([hid_dim, out_dim], edt, name="w2s")
            nc.sync.dma_start(out=w2s[:], in_=w2i[:, :])
            b1s = cp.tile([hid_dim, 1], f32, name="b1s")
            nc.sync.dma_start(out=b1s[:], in_=b1i[:, :])

            for t in range(tiles):
                K = int(Ks[t])
                if K == 0:
                    continue
                o0 = int(off[t])
                xg = gp.tile([128, kmax * 128], edt, name="xg", tag="xg")
                deng = nc.scalar if t % 2 == 0 else nc.sync
                deng.dma_start(
                    out=xg[:, :K * 128],
                    in_=xe[:, o0 * 128:(o0 + K) * 128])
                pB = pp.tile([128, 128], f32, name="pB", tag="acc")
                for g in range(K):
                    # h1T[h, d] += W1^T @ xgT_g   (W1 resident in PE)
                    nc.tensor.matmul(
                        out=pB[:], lhsT=w1s[:],
                        rhs=xg[:, g * 128:(g + 1) * 128],
                        start=(g == 0), stop=(g == K - 1))
                h1T = ep.tile([128, 128], edt, name="h1T", tag="h1T")
                nc.scalar.activation(out=h1T[:], in_=pB[:], func=Relu,
                                     bias=b1s[:, 0:1], scale=1.0)
                # h2T = W2^T @ h1T (b2 is folded into the phase-2 messages)
                pC = pp.tile([out_dim, 128], f32, name="pC", tag="finC")
                nc.tensor.matmul(out=pC[:], lhsT=w2s[:], rhs=h1T[:],
                                 start=True, stop=True)
                h2T = ep.tile([out_dim, 128], f32, name="h2T", tag="h2T")
                nc.vector.tensor_copy(out=h2T[:], in_=pC[:])
                oeng = nc.sync if t % 2 == 0 else nc.scalar
                oeng.dma_start(
                    out=h2o[t * out_dim:(t + 1) * out_dim, :], in_=h2T[:])
    nc.compile()
    return nc


def build_phase2(Ks, *, out_dim=OUT_DIM, n_cores=N_CORES):
    """h2-messages (pre-gathered, pre-scaled, +b2 folded) -> out = segsum."""
    import concourse.bacc as bacc
    import concourse.mybir as mybir
    import concourse.tile as tile

    f32 = mybir.dt.float32
    edt = _edge_dt()
    chk_total = int(sum(Ks))
    tiles = len(Ks)
    kmax = int(max(Ks))
    off = np.concatenate([[0], np.cumsum(Ks)]).astype(int)

    nc = bacc.Bacc("TRN2", target_bir_lowering=False, debug=False,
                   num_devices=n_cores)
    he = nc.dram_tensor("he", [128, chk_total * out_dim], edt,
                        kind="ExternalInput")
    ici = nc.dram_tensor("identc", [128, 128], edt, kind="ExternalInput")
    out_t = nc.dram_tensor("out", [tiles * 128, out_dim], f32,
                           kind="ExternalOutput")

    with tile.TileContext(nc) as tc:
        with (
            tc.tile_pool(name="const", bufs=1) as cp,
            tc.tile_pool(name="gath", bufs=3) as gp,
            tc.tile_pool(name="epi", bufs=3) as ep,
            tc.tile_pool(name="psum", bufs=2, space="PSUM") as pp,
        ):
            ics = cp.tile([128, 128], edt, name="ics")
            nc.sync.dma_start(out=ics[:], in_=ici[:, :])

            for t in range(tiles):
                K = int(Ks[t])
                if K == 0:
                    continue
                o0 = int(off[t])
                hg = gp.tile([128, kmax * out_dim], edt, name="hg", tag="hg")
                deng = nc.scalar if t % 2 == 0 else nc.sync
                deng.dma_start(
                    out=hg[:, :K * out_dim],
                    in_=he[:, o0 * out_dim:(o0 + K) * out_dim])
                pE = pp.tile([128, out_dim], f32, name="pE", tag="acc")
                for g in range(K):
                    # out[d, o] += hg_g[d, o]  (identity as resident weights)
                    nc.tensor.matmul(
                        out=pE[:], lhsT=ics[:],
                        rhs=hg[:, g * out_dim:(g + 1) * out_dim],
                        start=(g == 0), stop=(g == K - 1))
                ot = ep.tile([128, out_dim], f32, name="ot", tag="ot")
                nc.vector.tensor_copy(out=ot[:], in_=pE[:])
                oeng = nc.sync if t % 2 == 0 else nc.scalar
                oeng.dma_start(out=out_t[t * 128:(t + 1) * 128, :],
                               in_=ot[:])
    nc.compile()
    return nc


def shard_edges(edge_index, *, n_nodes=N_NODES, n_cores=N_CORES):
    """Degree-sorted node packing + per-(core,tile,chunk) edge slotting."""
    ei = np.asarray(edge_index)
    src = ei[0].astype(np.int64)
    dst = ei[1].astype(np.int64)

    loops = np.arange(n_nodes, dtype=np.int64)
    src_all = np.concatenate([src, loops])
    dst_all = np.concatenate([dst, loops])
    e_tot = src_all.shape[0]

    deg = np.bincount(dst_all, minlength=n_nodes).astype(np.int64)
    dis = 1.0 / np.sqrt(deg.astype(np.float64))  # self-loops => deg >= 1
    norm = (dis[src_all] * dis[dst_all]).astype(np.float32)

    n_blocks = math.ceil(n_nodes / 128)
    n_groups = math.ceil(n_blocks / n_cores)

    srt = np.argsort(-deg, kind="stable")  # node ranks by degree desc
    rank = np.empty(n_nodes, np.int64)
    rank[srt] = np.arange(n_nodes)
    node_block = rank // 128
    node_row = rank % 128
    node_core = node_block % n_cores
    node_tile = node_block // n_cores

    # per-tile chunk count = max degree in the tile group (same on all cores)
    Ks = np.zeros(n_groups, np.int64)
    deg_sorted = deg[srt]
    for t in range(n_groups):
        lo = t * n_cores * 128
        if lo < n_nodes:
            Ks[t] = deg_sorted[lo]  # degree-desc => group max is first entry
    off = np.concatenate([[0], np.cumsum(Ks)]).astype(np.int64)
    chk_total = int(off[-1])

    # edge slot: chunk g = rank within dst's edge list, partition = node row
    e_core = node_core[dst_all]
    e_tile = node_tile[dst_all]
    e_row = node_row[dst_all]
    order = np.argsort(dst_all, kind="stable")
    counts = np.bincount(dst_all, minlength=n_nodes)
    starts = np.concatenate([[0], np.cumsum(counts)])[:-1]
    g_of = np.empty(e_tot, np.int64)
    g_of[order] = np.arange(e_tot) - starts[dst_all[order]]

    slot = (off[e_tile] + g_of) * 128 + e_row  # within the core's array
    first = g_of == 0

    return {
        "Ks": Ks,
        "chk_total": chk_total,
        "src": src_all,
        "norm": norm,
        "slot": slot,
        "core": e_core,
        "first": first,
        "node_core": node_core,
        "node_tile": node_tile,
        "node_row": node_row,
    }


def edge_payload(shard, table, c, *, bias=None, transposed=False):
    """Per-core edge-message array. slot (t,g,p) holds table[src]*norm
    (+bias on the first slot of each dst segment). transposed=True lays each
    chunk out as [k, d] (phase 1); otherwise [d, k] blocks (phase 2)."""
    D = table.shape[1]
    chk_total = shard["chk_total"]
    m = shard["core"] == c
    arr = np.zeros((chk_total * 128, D), dtype=np.float32)
    vals = table[shard["src"][m]] * shard["norm"][m][:, None]
    if bias is not None:
        fm = shard["first"][m]
        vals[fm] += bias.reshape(1, -1)
    arr[shard["slot"][m]] = vals
    if transposed:
        assert D == 128
        return np.ascontiguousarray(arr.T).astype(_edge_np())
    out = arr.reshape(chk_total, 128, D).transpose(1, 0, 2)
    return np.ascontiguousarray(out.reshape(128, chk_total * D)).astype(
        _edge_np())


def gather_rows(shard, results, name, *, n_nodes=N_NODES,
                transposed=False):
    """Undo the node permutation (transposed=True for [tiles*dim, 128])."""
    cores = shard["node_core"]
    if transposed:
        arrs = []
        for r in results:
            a = r[name]  # [tiles*OUT_DIM, 128] tile-major transposed
            tiles = a.shape[0] // OUT_DIM
            arrs.append(a.reshape(tiles, OUT_DIM, 128).transpose(0, 2, 1)
                        .reshape(tiles * 128, OUT_DIM))
        results = [{name: a} for a in arrs]
    rows = shard["node_tile"] * 128 + shard["node_row"]
    dim = results[0][name].shape[1]
    out = np.empty((n_nodes, dim), dtype=np.float32)
    for c in range(len(results)):
        m = cores == c
        out[m] = results[c][name][rows[m]]
    return out


def _log(msg):
    if os.environ.get("BASSGCN_VERBOSE", "0") == "1":
        import time
        print(f"[bassgcn +{time.time() % 100000:.1f}] {msg}", flush=True)


def kernel(x, edge_index, W1, b1, W2, b2):
    global LAST_RESULTS
    from concourse.bass_utils import run_bass_kernel_spmd

    trace = os.environ.get("BASSGCN_TRACE", "0") == "1"
    if trace:
        _install_axon_ntff_hook()

    x = np.ascontiguousarray(np.asarray(x, dtype=np.float32))
    W1 = np.ascontiguousarray(np.asarray(W1, dtype=np.float32))
    W2 = np.ascontiguousarray(np.asarray(W2, dtype=np.float32))
    b1 = np.asarray(b1, dtype=np.float32).reshape(-1, 1)
    b2 = np.asarray(b2, dtype=np.float32).reshape(-1)

    _log("preprocess start")
    shard = shard_edges(edge_index)
    Ks = shard["Ks"]
    _log(f"shard done chunks={shard['chk_total']} kmax={int(Ks.max())}")
    identc = np.eye(128, dtype=np.float32).astype(_edge_np())
    W1h = W1.astype(_edge_np())

    nc1 = build_phase1(Ks)
    _log("phase1 built+compiled")
    in_maps1 = []
    for c in range(N_CORES):
        in_maps1.append({
            "xe": edge_payload(shard, x, c, transposed=True),
            "W1h": W1h, "W2h": W2.astype(_edge_np()), "b1": b1,
        })
    _log("phase1 payloads ready")
    res1 = run_bass_kernel_spmd(nc1, in_maps1, core_ids=list(range(N_CORES)),
                                trace=trace)
    _log("phase1 ran")

    h2 = gather_rows(shard, [res1.results[c] for c in range(N_CORES)], "h2",
                     transposed=True)

    nc2 = build_phase2(Ks)
    _log("phase2 built+compiled")
    in_maps2 = []
    for c in range(N_CORES):
        in_maps2.append({
            "he": edge_payload(shard, h2, c, bias=b2),
            "identc": identc,
        })
    _log("phase2 payloads ready")
    res2 = run_bass_kernel_spmd(nc2, in_maps2, core_ids=list(range(N_CORES)),
                                trace=trace)
    _log("phase2 ran")
    LAST_RESULTS = [res1, res2]

    out = gather_rows(shard, [res2.results[c] for c in range(N_CORES)], "out")
    return out.astype(np.float32)
